# revision 19
# baseline (speedup 1.0000x reference)
"""LoRA Linear kernel for Trainium2, 8 NeuronCores.

Computes out = x @ (W + lora_A @ lora_B)^T + bias for
x [4, 2048, 4096], W [4096, 4096], lora_A [4096, 16], lora_B [16, 4096].

Sharding: 2-way over tokens (M = 8192 -> 4096/core) x 4-way over
out_features (4096 -> 1024/core). The LoRA delta is folded into W on the
host (rank-16, negligible), so the device kernel is a pure streaming GEMM
in bf16 with fp32 PSUM accumulation. The schedule is a wavefront: the
static instruction order mirrors the DMA arrival order so the PE starts
within a few us and never head-of-line blocks on the 8 MiB weight
stream. Outputs are written back as bf16 (upcast on host) to halve store
traffic.

Waves:
  1) tiles 0-7, out-half 0 only (1 PSUM bank each = 8 banks): tiles join
     the ki-interleave as their x lands; W half 0 streams in ki order.
  2) tiles 0-7, out-half 1 (W half 1 streamed behind half 0).
  3) tiles 8-31, both halves per tile, 4-deep PSUM pipeline; x tiles
     prefetched 6 deep.
"""

import ml_dtypes

import numpy as np

import concourse.bass as bass
import concourse.bacc as bacc
import concourse.mybir as mybir
import concourse.tile as tile
from concourse.bass_utils import run_bass_kernel_spmd

IN_F = 4096
OUT_F = 4096
RANK = 16
BATCH, SEQ = 4, 2048
M_TOT = BATCH * SEQ          # 8192 tokens
MG, OG = 2, 4                # shard grid: token-groups x outfeature-groups
M_LOC = M_TOT // MG          # 4096 tokens per core
O_LOC = OUT_F // OG          # 1024 out features per core
P = 128
KI = IN_F // P               # 32 contraction tiles
NF = 512                     # matmul moving free dim (one PSUM bank)
OS = O_LOC // NF             # 2 output column halves
MT = M_LOC // P              # 32 token tiles per core
NLEAD = 8                    # tiles in waves 1+2
XSLOTS = 14                  # SBUF x-tile slots

F32 = mybir.dt.float32
BF16 = mybir.dt.bfloat16

_cache = {}


def _build():
    nc = bacc.Bacc(None, target_bir_lowering=False)

    # x pre-tiled on host to [MT, P, KI, P]: (mt, i_within, i_tile, m)
    xt = nc.dram_tensor("xt", [MT, P, KI, P], BF16, kind="ExternalInput")
    # W^T (with LoRA delta folded) laid out partition-major [P, KI, OS, NF]:
    # any ki-range chunk then matches the SBUF destination element order.
    wt = nc.dram_tensor("wt", [P, KI, OS, NF], BF16, kind="ExternalInput")
    br = nc.dram_tensor("br", [P, O_LOC], F32, kind="ExternalInput")
    out = nc.dram_tensor("out", [MT, P, OS, NF], BF16, kind="ExternalOutput")

    with tile.TileContext(nc) as tc:
        with (
            tc.tile_pool(name="const", bufs=1) as const_pool,
            tc.tile_pool(name="xin", bufs=XSLOTS) as xin_pool,
            tc.tile_pool(name="outs", bufs=8) as out_pool,
            tc.tile_pool(name="psum", bufs=8, space="PSUM") as psum_pool,
        ):
            wtot = const_pool.tile([P, KI, OS, NF], BF16, name="wtot")
            bias_sb = const_pool.tile([P, O_LOC], F32, name="bias_sb")

            engs = [nc.scalar, nc.sync, nc.gpsimd]
            qi = [0]

            def nxt():
                e = engs[qi[0] % 3]
                qi[0] += 1
                return e

            def dma_w(k, os_):
                nxt().dma_start(wtot[:, k, os_, :], wt[k, os_])

            x_tiles = {}

            def load_x(t, bounds=(0, KI), engs_override=None):
                if t not in x_tiles:
                    x_tiles[t] = xin_pool.tile([P, KI, P], BF16, name=f"x{t}", tag="x")
                tl = x_tiles[t]
                a, b = bounds
                e = engs_override if engs_override is not None else nxt()
                e.dma_start(tl[:, a:b, :], xt[t, :, a:b, :])

            ps = {}

            def mmp(t, k):
                # paired matmuls: same stationary x slice for both out halves
                # (consecutive identical lhsT keeps the PE at the 216 ns/MM
                # issue floor; alternating lhsT costs ~+43 ns/MM).
                for os_ in range(OS):
                    if k == 0:
                        ps[(t, os_)] = psum_pool.tile(
                            [P, NF], F32, name=f"ps{t}_{os_}", tag="ps"
                        )
                    nc.tensor.matmul(
                        ps[(t, os_)][:],
                        x_tiles[t][:, k, :],
                        wtot[:, k, os_, :],
                        start=(k == 0),
                        stop=(k == KI - 1),
                    )

            def store(t, os_, hwdge_only=False):
                o_tile = out_pool.tile([P, NF], BF16, name="o_tile", tag="o")
                nc.vector.tensor_add(
                    out=o_tile[:],
                    in0=ps[(t, os_)][:],
                    in1=bias_sb[:, os_ * NF : (os_ + 1) * NF],
                )
                del ps[(t, os_)]
                if hwdge_only:
                    e = engs[qi[0] % 2]  # scalar / sync only
                    qi[0] += 1
                else:
                    e = nxt()
                e.dma_start(out[t, :, os_, :], o_tile[:])

            # ---- wave 1: tiles 0..3 (2 PSUM banks each), wavefront joins ----
            # Explicit per-queue DMA programs, sized so each queue's FIFO
            # serves operands in the order the PE consumes them: x1-x3 early
            # (tile joins), W in 512 KiB 2-ki chunks spread over all three
            # queues slightly ahead of the PE's ki frontier.
            S, Y, G = nc.scalar, nc.sync, nc.gpsimd

            def dma_wr(a, b, eng):
                eng.dma_start(wtot[:, a:b, :, :], wt[:, a:b, :, :])

            # per-queue programs (FIFO order = service order):
            # S: x0c0, x2, W[6:10), W[18:22), x5, x8
            # Y: W[0:1), W[1:3), x1, W[10:14), W[22:26), x4, x7
            # G: x0c1, W[3:6), x3, W[14:18), W[26:30), W[30:32), bias, x6, x9
            load_x(0, (0, 4), S)
            dma_wr(0, 1, Y)
            load_x(0, (4, KI), G)
            dma_wr(1, 3, Y)
            mmp(0, 0)
            load_x(2, (0, KI), S)
            load_x(1, (0, KI), Y)
            dma_wr(3, 6, G)
            mmp(0, 1)
            mmp(0, 2)
            for k in range(3):
                mmp(1, k)  # t1 catch-up
            dma_wr(6, 10, S)
            load_x(3, (0, KI), G)
            for k in range(3, 6):
                for t in range(2):
                    mmp(t, k)
            for k in range(6):
                mmp(2, k)  # t2 catch-up
            dma_wr(10, 14, Y)
            dma_wr(14, 18, G)
            for k in range(6, 10):
                for t in range(3):
                    mmp(t, k)
            for k in range(10):
                mmp(3, k)  # t3 catch-up
            dma_wr(18, 22, S)
            dma_wr(22, 26, Y)
            dma_wr(26, 30, G)
            for k in range(10, 18):
                for t in range(4):
                    mmp(t, k)
            dma_wr(30, KI, G)
            G.dma_start(bias_sb[:], br[:])
            load_x(4, (0, KI), Y)
            load_x(5, (0, KI), S)
            for k in range(18, KI):
                for t in range(4):
                    mmp(t, k)
            load_x(6, (0, KI), G)
            load_x(7, (0, KI), Y)
            load_x(8, (0, KI), S)
            load_x(9, (0, KI), G)
            for t in range(4):
                store(t, 0)
                store(t, 1)

            # ---- wave 2: tiles 4..31, steady state, x prefetch 5 deep ----
            for t in range(4, MT):
                if t + 6 < MT:
                    load_x(t + 6)
                late = t >= MT - 3
                for k in range(KI):
                    mmp(t, k)
                store(t, 0, hwdge_only=late)
                store(t, 1, hwdge_only=late)
    nc.finalize()
    return nc


def kernel(x, W, bias, lora_A, lora_B):
    x = np.asarray(x, dtype=np.float32)
    W = np.asarray(W, dtype=np.float32)
    bias = np.asarray(bias, dtype=np.float32)
    lora_A = np.asarray(lora_A, dtype=np.float32)
    lora_B = np.asarray(lora_B, dtype=np.float32)

    if "nc" not in _cache:
        _cache["nc"] = _build()
    nc = _cache["nc"]

    Wtot = W + lora_A @ lora_B  # fold the rank-16 LoRA delta on host

    xr = x.reshape(M_TOT, IN_F).astype(ml_dtypes.bfloat16)
    xs_by_mg = []
    for mg in range(MG):
        xs = xr[mg * M_LOC : (mg + 1) * M_LOC]
        # [M_LOC, IN_F] -> (mt, m, ki, p) -> (mt, p, ki, m)
        xs_by_mg.append(
            np.ascontiguousarray(xs.reshape(MT, P, KI, P).transpose(0, 3, 2, 1))
        )
    wt_by_og = []
    br_by_og = []
    for og in range(OG):
        wT = Wtot[og * O_LOC : (og + 1) * O_LOC].T.astype(ml_dtypes.bfloat16)
        # [IN_F, O_LOC] -> (ki, p, os, nf) -> (p, ki, os, nf)
        wt_by_og.append(
            np.ascontiguousarray(wT.reshape(KI, P, OS, NF).transpose(1, 0, 2, 3))
        )
        br_by_og.append(
            np.ascontiguousarray(
                np.broadcast_to(bias[og * O_LOC : (og + 1) * O_LOC], (P, O_LOC)).astype(
                    np.float32
                )
            )
        )

    in_maps = []
    for c in range(8):
        mg, og = c % MG, c // MG
        in_maps.append(
            {"xt": xs_by_mg[mg], "wt": wt_by_og[og], "br": br_by_og[og]}
        )

    res = run_bass_kernel_spmd(nc, in_maps, core_ids=list(range(8)))

    out = np.empty((M_TOT, OUT_F), dtype=np.float32)
    for c in range(8):
        mg, og = c % MG, c // MG
        # [MT, P, OS, NF] -> rows (mt,m), cols (os,nf)
        blk = np.asarray(res.results[c]["out"]).reshape(M_LOC, O_LOC)
        out[mg * M_LOC : (mg + 1) * M_LOC, og * O_LOC : (og + 1) * O_LOC] = blk.astype(
            np.float32
        )
    return out.reshape(BATCH, SEQ, OUT_F)
